# revision 1
# baseline (speedup 1.0000x reference)
"""Trainium2 Bass kernel for nn_CountingLoss.

Computes, for pred (16,2,1024,1024) f32 and target (16,1024,1024) f32:
  seg_loss   = mean pixelwise 2-class softmax CE
  count_loss = mean_b |count(pred_b) - count(target_b)|
where count() = number of distinct nonzero labels after a 32-iteration
masked 3x3 max-pool flood-fill CCL seeded with raster iota labels.

Distinct-count trick (exact): a label value v = init[q] survives in the
final label map L iff  min{L[p] : p in graph-ball(q,32)} == init[q].
That min-flood is the same masked max-pool flood applied to (K - L).
So: 32 max-flood iters + 32 min-flood iters + elementwise compare/reduce.

Sharding: pure data parallel, 2 samples per core across 8 NeuronCores.
Per-core outputs: [seg_sum_s0, seg_sum_s1, tcnt0, tcnt1, pcnt0, pcnt1, 0, 0];
final means are combined on the host.
"""

import os
import numpy as np

H = 1024
W = 1024
B = 16
NCORES = 8
SPC = B // NCORES          # samples per core
RPP = H // 128             # rows per SBUF partition
FD = RPP * W               # owned free-dim elements per partition
ITERS = int(os.environ.get("BASS_CCL_ITERS", "32"))
KBIG = float(2 ** 21)

_built = {}


def _build(iters, bench=False, split=0):
    import contextlib
    import concourse.bass as bass  # noqa: F401
    import concourse.bacc as bacc
    import concourse.mybir as mybir
    import concourse.tile as tile

    fp = mybir.dt.float32
    Alu = mybir.AluOpType
    Act = mybir.ActivationFunctionType
    AX = mybir.AxisListType.X

    nc = bacc.Bacc("TRN2", target_bir_lowering=False, debug=False,
                   num_devices=NCORES)

    ishape = [1, 1] if bench else None
    pred_d = nc.dram_tensor("pred", ishape or [SPC, 2, H, W], fp,
                            kind="ExternalInput")
    tgt_d = nc.dram_tensor("target", ishape or [SPC, H, W], fp,
                            kind="ExternalInput")
    out_d = nc.dram_tensor("out", [8], fp, kind="ExternalOutput")

    def slab(ap2d):
        # [1024, 1024] DRAM view -> [128, FD] (partition p holds rows 8p..8p+7)
        return ap2d.rearrange("(p a) b -> p (a b)", p=128)

    with tile.TileContext(nc) as tc:
        with tc.tile_pool(name="main", bufs=1) as pool, \
             tc.tile_pool(name="ps", bufs=1, space="PSUM") as pspool:

            racc = pool.tile([128, 8], fp, tag="racc")
            red1 = pool.tile([128, 64], fp, tag="red1")
            ones = pool.tile([128, 1], fp, tag="ones")
            nc.gpsimd.memset(racc[:], 0.0)
            nc.gpsimd.memset(ones[:], 1.0)

            # ---------------- segmentation CE loss ----------------
            for s in range(SPC if not bench else 0):
                p0 = pool.tile([128, FD], fp, tag="A")
                p1 = pool.tile([128, FD], fp, tag="B")
                tg = pool.tile([128, FD], fp, tag="C")
                dd = pool.tile([128, FD], fp, tag="D")
                nc.sync.dma_start(p0[:], slab(pred_d[s, 0]))
                nc.sync.dma_start(p1[:], slab(pred_d[s, 1]))
                nc.sync.dma_start(tg[:], slab(tgt_d[s]))
                # d = p0 - p1
                nc.vector.tensor_tensor(dd[:], p0[:], p1[:], op=Alu.subtract)
                # tg <- (tg > 0.5) * d
                nc.vector.scalar_tensor_tensor(
                    tg[:], tg[:], 0.5, dd[:], op0=Alu.is_gt, op1=Alu.mult)
                # p0 <- relu(-d)  == max(p0,p1) - p0
                nc.scalar.activation(p0[:], dd[:], Act.Relu, scale=-1.0)
                # dd <- softplus(-|d|) == log(1 + exp(-|d|))
                nc.scalar.activation(dd[:], dd[:], Act.Abs)
                nc.scalar.activation(dd[:], dd[:], Act.Exp, scale=-1.0)
                nc.scalar.activation(dd[:], dd[:], Act.Ln, bias=1.0)
                # p0 <- relu(-d) + softplus(-|d|) + t*d   (pixel CE)
                nc.vector.tensor_tensor(p0[:], p0[:], dd[:], op=Alu.add)
                nc.vector.tensor_tensor(p0[:], p0[:], tg[:], op=Alu.add)
                # two-stage sum -> racc[:, s]
                nc.vector.reduce_sum(
                    red1[:, 0:64],
                    p0[:].rearrange("p (a b) -> p a b", b=128), axis=AX)
                nc.vector.reduce_sum(racc[:, s:s + 1], red1[:, 0:64], axis=AX)

            # ---------------- CCL counting floods ----------------
            # images: (slot, dram slab) -- counts go to racc[:, slot]
            images = []
            if bench:
                images = [(2 + i, None) for i in range(2 * SPC)]
            else:
                for s in range(SPC):
                    images.append((2 + s, slab(tgt_d[s])))
                for s in range(SPC):
                    images.append((2 + SPC + s, slab(pred_d[s, 1])))

            for slot, src in images:
                raw = pool.tile([128, FD], fp, tag="D")
                if not bench:
                    nc.sync.dma_start(raw[:], src)
                fg = pool.tile([128, FD], fp, tag="C")
                nc.vector.tensor_single_scalar(fg[:], raw[:], 0.5, op=Alu.is_gt)
                iota = pool.tile([128, FD], fp, tag="D")
                nc.gpsimd.iota(iota[:], pattern=[[1, FD]], base=0,
                               channel_multiplier=FD,
                               allow_small_or_imprecise_dtypes=True)
                S = pool.tile([128, FD], fp, tag="A")
                hh = pool.tile([128, FD], fp, tag="B")
                ht = pool.tile([128, W], fp, tag="ht")
                hb = pool.tile([128, W], fp, tag="hb")
                nc.gpsimd.memset(ht[:], 0.0)
                nc.gpsimd.memset(hb[:], 0.0)
                # S0 = iota * fg
                nc.vector.tensor_tensor(S[:], iota[:], fg[:], op=Alu.mult)

                S3 = S[:].rearrange("p (j x) -> p j x", x=W)
                h3 = hh[:].rearrange("p (j x) -> p j x", x=W)

                for phase in range(2):
                    if phase == 1:
                        # S <- (K - S) * fg   (min-flood encoding)
                        nc.vector.tensor_scalar(
                            S[:], S[:], -1.0, KBIG, op0=Alu.mult, op1=Alu.add)
                        nc.vector.tensor_tensor(S[:], S[:], fg[:], op=Alu.mult)
                    def btt(d, dsl, a, asl, b, bsl, op):
                        if split:
                            nc.vector.tensor_tensor(
                                d[0:split, dsl], a[0:split, asl],
                                b[0:split, bsl], op=op)
                            nc.gpsimd.tensor_tensor(
                                d[split:128, dsl], a[split:128, asl],
                                b[split:128, bsl], op=op)
                        else:
                            nc.vector.tensor_tensor(
                                d[:, dsl], a[:, asl], b[:, bsl], op=op)

                    SA = slice(0, FD)
                    for _ in range(iters):
                        # H-pass: hh = hmax3(S) along x (row-wise)
                        btt(hh, slice(1, FD - 1), S, slice(0, FD - 2),
                            S, slice(2, FD), Alu.max)
                        btt(hh, SA, hh, SA, S, SA, Alu.max)
                        # row-edge patches (x=0 and x=W-1 of each row)
                        nc.vector.tensor_tensor(
                            h3[:, :, 0:1], S3[:, :, 0:1], S3[:, :, 1:2],
                            op=Alu.max)
                        nc.vector.tensor_tensor(
                            h3[:, :, W - 1:W], S3[:, :, W - 2:W - 1],
                            S3[:, :, W - 1:W], op=Alu.max)
                        # halo rows of hh to neighbor partitions
                        nc.sync.dma_start(ht[1:128, :], hh[0:127, FD - W:FD])
                        nc.sync.dma_start(hb[0:127, :], hh[1:128, 0:W])
                        # V-pass: S = max(hh[y-1], hh[y+1]) piecewise
                        btt(S, slice(W, FD - W), hh, slice(0, FD - 2 * W),
                            hh, slice(2 * W, FD), Alu.max)
                        nc.vector.tensor_tensor(
                            S[:, 0:W], ht[:], hh[:, W:2 * W], op=Alu.max)
                        nc.vector.tensor_tensor(
                            S[:, FD - W:FD], hh[:, FD - 2 * W:FD - W], hb[:],
                            op=Alu.max)
                        btt(S, SA, S, SA, hh, SA, Alu.max)
                        # mask
                        if split:
                            btt(S, SA, S, SA, fg, SA, Alu.mult)
                        else:
                            nc.gpsimd.tensor_tensor(S[:], S[:], fg[:],
                                                    op=Alu.mult)

                # survive = (K - S == iota), excluding pixel (0,0)
                nc.vector.tensor_scalar(
                    S[:], S[:], -1.0, KBIG, op0=Alu.mult, op1=Alu.add)
                nc.vector.tensor_tensor(S[:], S[:], iota[:], op=Alu.is_equal)
                nc.vector.memset(S[0:1, 0:1], 0.0)
                nc.vector.reduce_sum(
                    red1[:, 0:64],
                    S[:].rearrange("p (a b) -> p a b", b=128), axis=AX)
                nc.vector.reduce_sum(racc[:, slot:slot + 1], red1[:, 0:64],
                                     axis=AX)

            # ---------------- partition reduce + output ----------------
            pt = pspool.tile([8, 1], fp)
            nc.tensor.matmul(pt[:], racc[:], ones[:], start=True, stop=True)
            oc = pool.tile([8, 1], fp, tag="oc")
            nc.scalar.copy(oc[:], pt[:])
            nc.sync.dma_start(out_d[:], oc[:])

    nc.compile()
    return nc


def _get_nc(iters, bench=False, split=0):
    key = (iters, bench, split)
    if key not in _built:
        _built[key] = _build(iters, bench=bench, split=split)
    return _built[key]


def run_cores(pred, target, iters=ITERS, trace=False, bench=False, split=0):
    from concourse import bass_utils
    from concourse.bass_interp import get_hw_module

    nc = _get_nc(iters, bench=bench, split=split)
    if bench:
        z = np.zeros((1, 1), np.float32)
        in_maps = [{"pred": z, "target": z} for _ in range(NCORES)]
    else:
        pred = np.ascontiguousarray(pred, np.float32)
        target = np.ascontiguousarray(target, np.float32)
        in_maps = [
            {"pred": pred[SPC * c:SPC * (c + 1)],
             "target": target[SPC * c:SPC * (c + 1)]}
            for c in range(NCORES)
        ]
    old = nc.m
    nc.m = get_hw_module(nc.m)
    try:
        res = bass_utils.run_bass_kernel_spmd(
            nc, in_maps, core_ids=list(range(NCORES)), trace=trace)
    finally:
        nc.m = old
    return res


def kernel(pred, target):
    res = run_cores(pred, target)
    outs = np.stack([r["out"] for r in res.results])  # [8, 8]
    seg_sum = float(outs[:, 0:SPC].sum(dtype=np.float64))
    seg_loss = np.float32(seg_sum / (B * H * W))
    tc = outs[:, 2:2 + SPC].reshape(-1)
    pc = outs[:, 2 + SPC:2 + 2 * SPC].reshape(-1)
    count_loss = np.float32(np.abs(pc - tc).mean(dtype=np.float64))
    return (seg_loss, count_loss)



# revision 3
# speedup vs baseline: 405.3530x; 405.3530x over previous
"""Trainium2 Bass kernel for nn_CountingLoss.

Computes, for pred (16,2,1024,1024) f32 and target (16,1024,1024) f32:
  seg_loss   = mean pixelwise 2-class softmax CE
  count_loss = mean_b |count(pred_b) - count(target_b)|
where count() = number of distinct nonzero labels after a 32-iteration
masked 3x3 max-pool flood-fill CCL seeded with raster iota labels.

Distinct-count trick (exact): a label value v = init[q] survives in the
final label map L iff  min{L[p] : p in graph-ball(q,32)} == init[q].
That min-flood is the same masked max-pool flood applied to (K - L).
So: 32 max-flood iters + 32 min-flood iters + elementwise compare/reduce.

Sharding: pure data parallel, 2 samples per core across 8 NeuronCores.
Per-core outputs: [seg_sum_s0, seg_sum_s1, tcnt0, tcnt1, pcnt0, pcnt1, 0, 0];
final means are combined on the host.
"""

import os
import numpy as np

H = 1024
W = 1024
B = 16
NCORES = 8
SPC = B // NCORES          # samples per core
RPP = H // 128             # rows per SBUF partition
FD = RPP * W               # owned free-dim elements per partition
ITERS = int(os.environ.get("BASS_CCL_ITERS", "32"))
KBIG = float(2 ** 21)

_built = {}


def _build(iters, bench=False, split=0):
    import contextlib
    import concourse.bass as bass  # noqa: F401
    import concourse.bacc as bacc
    import concourse.mybir as mybir
    import concourse.tile as tile

    fp = mybir.dt.float32
    Alu = mybir.AluOpType
    Act = mybir.ActivationFunctionType
    AX = mybir.AxisListType.X

    nc = bacc.Bacc("TRN2", target_bir_lowering=False, debug=False,
                   num_devices=NCORES)

    ishape = [1, 1] if bench else None
    pred_d = nc.dram_tensor("pred", ishape or [SPC, 2, H, W], fp,
                            kind="ExternalInput")
    tgt_d = nc.dram_tensor("target", ishape or [SPC, H, W], fp,
                            kind="ExternalInput")
    out_d = nc.dram_tensor("out", [8], fp, kind="ExternalOutput")

    def slab(ap2d):
        # [1024, 1024] DRAM view -> [128, FD] (partition p holds rows 8p..8p+7)
        return ap2d.rearrange("(p a) b -> p (a b)", p=128)

    with tile.TileContext(nc) as tc:
        with tc.tile_pool(name="main", bufs=1) as pool, \
             tc.tile_pool(name="ps", bufs=1, space="PSUM") as pspool:

            racc = pool.tile([128, 8], fp, tag="racc")
            red1 = pool.tile([128, 64], fp, tag="red1")
            ones = pool.tile([128, 1], fp, tag="ones")
            nc.gpsimd.memset(racc[:], 0.0)
            nc.gpsimd.memset(ones[:], 1.0)

            # ---------------- segmentation CE loss ----------------
            for s in range(SPC if not bench else 0):
                p0 = pool.tile([128, FD], fp, tag="A")
                p1 = pool.tile([128, FD], fp, tag="B")
                tg = pool.tile([128, FD], fp, tag="C")
                dd = pool.tile([128, FD], fp, tag="D")
                nc.sync.dma_start(p0[:], slab(pred_d[s, 0]))
                nc.sync.dma_start(p1[:], slab(pred_d[s, 1]))
                nc.sync.dma_start(tg[:], slab(tgt_d[s]))
                # d = p0 - p1
                nc.vector.tensor_tensor(dd[:], p0[:], p1[:], op=Alu.subtract)
                # tg <- (tg > 0.5) * d
                nc.vector.scalar_tensor_tensor(
                    tg[:], tg[:], 0.5, dd[:], op0=Alu.is_gt, op1=Alu.mult)
                # p0 <- relu(-d)  == max(p0,p1) - p0
                nc.scalar.activation(p0[:], dd[:], Act.Relu, scale=-1.0)
                # dd <- softplus(-|d|) == log(1 + exp(-|d|))
                nc.scalar.activation(dd[:], dd[:], Act.Abs)
                nc.scalar.activation(dd[:], dd[:], Act.Exp, scale=-1.0)
                nc.scalar.activation(dd[:], dd[:], Act.Ln, bias=1.0)
                # p0 <- relu(-d) + softplus(-|d|) + t*d   (pixel CE)
                nc.vector.tensor_tensor(p0[:], p0[:], dd[:], op=Alu.add)
                nc.vector.tensor_tensor(p0[:], p0[:], tg[:], op=Alu.add)
                # two-stage sum -> racc[:, s]
                nc.vector.reduce_sum(
                    red1[:, 0:64],
                    p0[:].rearrange("p (a b) -> p a b", b=128), axis=AX)
                nc.vector.reduce_sum(racc[:, s:s + 1], red1[:, 0:64], axis=AX)

            # ---------------- CCL counting floods ----------------
            # images: (slot, dram slab) -- counts go to racc[:, slot]
            images = []
            if bench:
                images = [(2 + i, None) for i in range(2 * SPC)]
            else:
                for s in range(SPC):
                    images.append((2 + s, slab(tgt_d[s])))
                for s in range(SPC):
                    images.append((2 + SPC + s, slab(pred_d[s, 1])))

            for slot, src in images:
                raw = pool.tile([128, FD], fp, tag="D")
                if not bench:
                    nc.sync.dma_start(raw[:], src)
                fg = pool.tile([128, FD], fp, tag="C")
                nc.vector.tensor_single_scalar(fg[:], raw[:], 0.5, op=Alu.is_gt)
                iota = pool.tile([128, FD], fp, tag="D")
                nc.gpsimd.iota(iota[:], pattern=[[1, FD]], base=0,
                               channel_multiplier=FD,
                               allow_small_or_imprecise_dtypes=True)
                S = pool.tile([128, FD], fp, tag="A")
                hh = pool.tile([128, FD], fp, tag="B")
                ht = pool.tile([128, W], fp, tag="ht")
                hb = pool.tile([128, W], fp, tag="hb")
                nc.gpsimd.memset(ht[:], 0.0)
                nc.gpsimd.memset(hb[:], 0.0)
                # S0 = iota * fg
                nc.vector.tensor_tensor(S[:], iota[:], fg[:], op=Alu.mult)

                S3 = S[:].rearrange("p (j x) -> p j x", x=W)
                h3 = hh[:].rearrange("p (j x) -> p j x", x=W)

                for phase in range(2):
                    if phase == 1:
                        # S <- (K - S) * fg   (min-flood encoding)
                        nc.vector.tensor_scalar(
                            S[:], S[:], -1.0, KBIG, op0=Alu.mult, op1=Alu.add)
                        nc.vector.tensor_tensor(S[:], S[:], fg[:], op=Alu.mult)
                    def btt(d, dsl, a, asl, b, bsl, op):
                        if split:
                            nc.vector.tensor_tensor(
                                d[0:split, dsl], a[0:split, asl],
                                b[0:split, bsl], op=op)
                            nc.gpsimd.tensor_tensor(
                                d[split:128, dsl], a[split:128, asl],
                                b[split:128, bsl], op=op)
                        else:
                            nc.vector.tensor_tensor(
                                d[:, dsl], a[:, asl], b[:, bsl], op=op)

                    SA = slice(0, FD)
                    for _ in range(iters):
                        # H-pass: hh = hmax3(S) along x (row-wise)
                        btt(hh, slice(1, FD - 1), S, slice(0, FD - 2),
                            S, slice(2, FD), Alu.max)
                        btt(hh, SA, hh, SA, S, SA, Alu.max)
                        # row-edge patches (x=0 and x=W-1 of each row)
                        nc.vector.tensor_tensor(
                            h3[:, :, 0:1], S3[:, :, 0:1], S3[:, :, 1:2],
                            op=Alu.max)
                        nc.vector.tensor_tensor(
                            h3[:, :, W - 1:W], S3[:, :, W - 2:W - 1],
                            S3[:, :, W - 1:W], op=Alu.max)
                        # halo rows of hh to neighbor partitions
                        nc.sync.dma_start(ht[1:128, :], hh[0:127, FD - W:FD])
                        nc.sync.dma_start(hb[0:127, :], hh[1:128, 0:W])
                        # V-pass: S = max(hh[y-1], hh[y+1]) piecewise
                        btt(S, slice(W, FD - W), hh, slice(0, FD - 2 * W),
                            hh, slice(2 * W, FD), Alu.max)
                        nc.vector.tensor_tensor(
                            S[:, 0:W], ht[:], hh[:, W:2 * W], op=Alu.max)
                        nc.vector.tensor_tensor(
                            S[:, FD - W:FD], hh[:, FD - 2 * W:FD - W], hb[:],
                            op=Alu.max)
                        btt(S, SA, S, SA, hh, SA, Alu.max)
                        # mask
                        if split:
                            btt(S, SA, S, SA, fg, SA, Alu.mult)
                        else:
                            nc.gpsimd.tensor_tensor(S[:], S[:], fg[:],
                                                    op=Alu.mult)

                # survive = (K - S == iota), excluding pixel (0,0)
                nc.vector.tensor_scalar(
                    S[:], S[:], -1.0, KBIG, op0=Alu.mult, op1=Alu.add)
                nc.vector.tensor_tensor(S[:], S[:], iota[:], op=Alu.is_equal)
                nc.vector.memset(S[0:1, 0:1], 0.0)
                nc.vector.reduce_sum(
                    red1[:, 0:64],
                    S[:].rearrange("p (a b) -> p a b", b=128), axis=AX)
                nc.vector.reduce_sum(racc[:, slot:slot + 1], red1[:, 0:64],
                                     axis=AX)

            # ---------------- partition reduce + output ----------------
            pt = pspool.tile([8, 1], fp)
            nc.tensor.matmul(pt[:], racc[:], ones[:], start=True, stop=True)
            oc = pool.tile([8, 1], fp, tag="oc")
            nc.scalar.copy(oc[:], pt[:])
            nc.sync.dma_start(out_d[:], oc[:])

    nc.compile()
    return nc


def _get_nc(iters, bench=False, split=0):
    key = (iters, bench, split)
    if key not in _built:
        _built[key] = _build(iters, bench=bench, split=split)
    return _built[key]


def run_cores(pred, target, iters=ITERS, trace=False, bench=False, split=0,
              **kw):
    from concourse import bass_utils
    from concourse.bass_interp import get_hw_module

    nc = _get_nc(iters, bench=bench, split=split)
    if bench:
        z = np.zeros((1, 1), np.float32)
        in_maps = [{"pred": z, "target": z} for _ in range(NCORES)]
    else:
        pred = np.ascontiguousarray(pred, np.float32)
        target = np.ascontiguousarray(target, np.float32)
        in_maps = [
            {"pred": pred[SPC * c:SPC * (c + 1)],
             "target": target[SPC * c:SPC * (c + 1)]}
            for c in range(NCORES)
        ]
    old = nc.m
    nc.m = get_hw_module(nc.m)
    try:
        res = bass_utils.run_bass_kernel_spmd(
            nc, in_maps, core_ids=list(range(NCORES)), trace=trace, **kw)
    finally:
        nc.m = old
    return res


def kernel(pred, target):
    res = run_cores(pred, target)
    outs = np.stack([r["out"] for r in res.results])  # [8, 8]
    seg_sum = float(outs[:, 0:SPC].sum(dtype=np.float64))
    seg_loss = np.float32(seg_sum / (B * H * W))
    tc = outs[:, 2:2 + SPC].reshape(-1)
    pc = outs[:, 2 + SPC:2 + 2 * SPC].reshape(-1)
    count_loss = np.float32(np.abs(pc - tc).mean(dtype=np.float64))
    return (seg_loss, count_loss)

